# revision 54
# baseline (speedup 1.0000x reference)
"""AdaINResBlock1 (HiFi-GAN style) Trainium2 kernel, batch-parallel over 8 NeuronCores.

Layout: channels on partitions (4 groups x 128), time on the free axis.
Convs run as bf16 matmuls accumulating f32 in PSUM, emitted chunk-by-chunk
in T order so they pipeline against the snake producer; weight-norm scale
fuses into the DVE eviction and the sum-of-squares for the next instance
norm rides an ACT Square op per evicted chunk, so no separate stats pass
ever touches the full tensor. Weight prep for the next conv is emitted as
fillers inside the current conv to keep it off the critical DVE window.
"""

import math
import os
import sys
from contextlib import ExitStack

import numpy as np

try:
    import concourse.bass as bass
except ImportError:  # pragma: no cover
    sys.path.insert(0, "/opt/trn_rl_repo")
    import concourse.bass as bass

import concourse.tile as tile
from concourse import bacc, mybir

f32 = mybir.dt.float32
bf16 = mybir.dt.bfloat16
f16 = mybir.dt.float16
AF = mybir.ActivationFunctionType
OP = mybir.AluOpType
AX = mybir.AxisListType

B, T_FULL, C, S, KW = 8, 4096, 512, 64, 3
DILATIONS = (1, 3, 5)
EPS = 1e-5
G = C // 128          # 4 channel groups of 128 partitions
PADL = 5              # max dilation -> left/right zero pad for conv1 input
TCH = 512             # t-chunk width (one PSUM bank)
N_CORES = 8


def build_nc(T=T_FULL, max_alpha=1.0, n_iters=3):
    NT = T // TCH
    # snake chunk bounds: fine at the head so the conv can start early; the
    # 520-col chunk k covers everything conv chunk k needs (512k+512+5 <= 520(k+1))
    if T >= 4096:
        SNAKE_BOUNDS = [0, 520, 1040, 1560, 2080, 2600, 3120, 3640, T]
    else:
        SNAKE_BOUNDS = [0, T]
    SNK_MAX = max(b - a for a, b in zip(SNAKE_BOUNDS[:-1], SNAKE_BOUNDS[1:]))
    # ACT Sin is valid on [-pi, pi] only; each ADD_RANGE_WRAP pass unwraps one
    # period. Bound the angle by max_alpha * 9 (|a| <= (1+gamma)*|xn| + beta
    # stays well under 9 for instance-normalized activations).
    N_WRAPS = max(1, int(math.ceil((max_alpha * 9.0 - math.pi) / (2 * math.pi))))

    nc = bacc.Bacc()
    x_ext = nc.declare_dram_parameter("x", [C, T], f32, isOutput=False)
    s_ext = nc.declare_dram_parameter("s", [S, 1], f32, isOutput=False)
    fc1_w_ext = nc.declare_dram_parameter("fc1_w", [3, S, 2 * C], f32, isOutput=False)
    fc1_b_ext = nc.declare_dram_parameter("fc1_b", [3, 2 * C], f32, isOutput=False)
    alpha1_ext = nc.declare_dram_parameter("alpha1", [3, C], f32, isOutput=False)
    conv1_v_ext = nc.declare_dram_parameter("conv1_v", [3, KW, C, C], f32, isOutput=False)
    conv1_g_ext = nc.declare_dram_parameter("conv1_g", [3, C], f32, isOutput=False)
    conv1_b_ext = nc.declare_dram_parameter("conv1_b", [3, C], f32, isOutput=False)
    fc2_w_ext = nc.declare_dram_parameter("fc2_w", [3, S, 2 * C], f32, isOutput=False)
    fc2_b_ext = nc.declare_dram_parameter("fc2_b", [3, 2 * C], f32, isOutput=False)
    alpha2_ext = nc.declare_dram_parameter("alpha2", [3, C], f32, isOutput=False)
    conv2_v_ext = nc.declare_dram_parameter("conv2_v", [3, KW, C, C], f32, isOutput=False)
    conv2_g_ext = nc.declare_dram_parameter("conv2_g", [3, C], f32, isOutput=False)
    conv2_b_ext = nc.declare_dram_parameter("conv2_b", [3, C], f32, isOutput=False)
    out_ext = nc.declare_dram_parameter("out", [C, T], f32, isOutput=True)

    with tile.TileContext(nc) as tc, ExitStack() as ctx:
        persist = ctx.enter_context(tc.tile_pool(name="persist", bufs=1))
        wpool = ctx.enter_context(tc.tile_pool(name="wpool", bufs=1))
        stage = ctx.enter_context(tc.tile_pool(name="stage", bufs=4))
        scr = ctx.enter_context(tc.tile_pool(name="scr", bufs=2))
        small = ctx.enter_context(tc.tile_pool(name="small", bufs=2))
        psc = ctx.enter_context(tc.tile_pool(name="psc", bufs=6, space="PSUM"))
        psm = ctx.enter_context(tc.tile_pool(name="psm", bufs=2, space="PSUM"))

        # ------------- persistent state -------------
        ones_col = persist.tile([128, 1], bf16, name="ones_col")
        nc.gpsimd.memset(ones_col, 1.0)
        ident1 = persist.tile([1, 1], f32, name="ident1")
        nc.gpsimd.memset(ident1, 1.0)
        junk = persist.tile([128, 1], f32, name="junk")
        zero_col = persist.tile([128, 1], f32, name="zero_col")
        nc.gpsimd.memset(zero_col, 0.0)
        eps_col = persist.tile([128, 1], f32, name="eps_col")
        nc.gpsimd.memset(eps_col, EPS)
        sqjunk = persist.tile([128, TCH], bf16, name="sqjunk")
        sqjunk2 = persist.tile([128, TCH], bf16, name="sqjunk2")

        s_sb = persist.tile([S, 1], f32, name="s_sb")
        nc.gpsimd.dma_start(out=s_sb, in_=s_ext[:, :])

        x_cur, b1pad, cb2pad = [], [], []
        for g in range(G):
            xc = persist.tile([128, T], f32, name=f"x_cur_{g}")
            x_cur.append(xc)
            bp = persist.tile([128, PADL + T + PADL], bf16, name=f"b1pad_{g}")
            b1pad.append(bp)
            cp = persist.tile([128, 1 + T + 1], bf16, name=f"cb2pad_{g}")
            cb2pad.append(cp)

        def load_pcvec_all(name, ext):
            # DRAM (3, C) -> (128, 3, G) tile, channel c = g*128 + p
            t = persist.tile([128, 3, G], f32, name=name)
            nc.gpsimd.dma_start(
                out=t, in_=ext.rearrange("i (g p) -> p i g", p=128))
            return t

        # ------------- emitters -------------
        def emit_wprep_dma(i, which):
            """Stage the raw f32 conv weights from DRAM (12 tiles)."""
            vext = conv1_v_ext if which == 1 else conv2_v_ext
            st = []
            for k in range(KW):
                for ci in range(G):
                    st_v = stage.tile([128, TCH], f32, tag="vstg", bufs=10,
                                      name=f"vst_{i}_{which}_{k}_{ci}")
                    nc.sync.dma_start(
                        out=st_v, in_=vext[i, k, ci * 128:(ci + 1) * 128, :])
                    st.append(st_v)
            return st

        def emit_wprep_cast(i, which, st, lo, hi, W, wv_all):
            """Cast staged tiles [lo,hi) to bf16 stationary W and square them."""
            for idx in range(lo, hi):
                w = wpool.tile([128, TCH], bf16, tag=f"w{which}_{idx}",
                               name=f"w{which}_{i}_{idx}")
                nc.vector.tensor_copy(out=w, in_=st[idx])
                vsq = scr.tile([128, TCH], bf16, tag=f"vsq{idx}",
                               name=f"vsq_{i}_{which}_{idx}", bufs=1)
                nc.scalar.activation(out=vsq, in_=st[idx], func=AF.Square,
                                     bias=zero_col)
                W[idx] = w
                wv_all.append(vsq)

        def emit_wprep_norm(i, which, vsqs):
            """g/||v|| per output channel from the squared tiles."""
            normps = psm.tile([1, C], f32, tag="pm", name=f"norm_{i}_{which}")
            for n, vsq in enumerate(vsqs):
                nc.tensor.matmul(
                    normps, ones_col, vsq,
                    start=(n == 0), stop=(n == KW * G - 1))
            nrow = small.tile([1, C], f32, tag="nrow", name=f"nrow_{i}_{which}",
                              bufs=1)
            nc.scalar.activation(out=nrow, in_=normps, func=AF.Copy)
            ps_t = psm.tile([128, G], f32, tag="pm", name=f"wnt_{i}_{which}")
            for g in range(G):
                nc.tensor.matmul(
                    ps_t[:, g:g + 1], nrow[0:1, g * 128:(g + 1) * 128], ident1,
                    is_transpose=True, start=(g == 0), stop=(g == G - 1))
            wnS = small.tile([128, G], f32, tag=f"wns{which}",
                             name=f"wns_{i}_{which}", bufs=2)
            nc.scalar.activation(out=wnS, in_=ps_t, func=AF.Sqrt, bias=zero_col)
            nc.vector.reciprocal(wnS, wnS)
            nc.vector.tensor_tensor(out=wnS, in0=wnS, in1=g_t[(i, which)], op=OP.mult)
            return wnS

        def wprep_fillers(i, which):
            """Weight prep scheduled as fillers under a conv: stage DMAs issue
            immediately; casts/squares spread over conv chunks 1..6 (when the
            DVE/ACT only carry evictions); norm matmuls after chunk 6, by
            which point the staged weights have long since landed."""
            st = emit_wprep_dma(i, which)
            W = [None] * (KW * G)
            wv_all = []
            wnS_box = {}

            def filler(tj):
                if 1 <= tj <= 6:
                    lo = (tj - 1) * 2
                    emit_wprep_cast(i, which, st, lo, min(lo + 2, KW * G),
                                    W, wv_all)
                elif tj == 7:
                    wnS_box[0] = emit_wprep_norm(i, which, wv_all)
            return filler, W, wnS_box

        def emit_fc(i, which):
            wext = fc1_w_ext if which == 1 else fc2_w_ext
            bext = fc1_b_ext if which == 1 else fc2_b_ext
            # fcw rides the gpsimd queue: the sync queue may be head-blocked
            # by gated weight-stage DMAs, and stats (which need h) must not
            # wait behind those.
            fcw = stage.tile([S, 2 * C], f32, tag="fcw", name=f"fcw_{i}_{which}", bufs=1)
            nc.gpsimd.dma_start(out=fcw, in_=wext[i])
            fcb = small.tile([128, 2 * C // 128], f32, tag="fcb",
                             name=f"fcb_{i}_{which}", bufs=2)
            nc.gpsimd.dma_start(out=fcb, in_=bext[i].rearrange("(c p) -> p c", p=128))
            hps = psm.tile([128, 2 * C // 128], f32, tag="pm", name=f"hps_{i}_{which}")
            ncc = 2 * C // 128
            for cc in range(ncc):
                nc.tensor.matmul(
                    hps[:, cc:cc + 1],
                    fcw[:, cc * 128:(cc + 1) * 128],
                    s_sb,
                    start=(cc == 0), stop=(cc == ncc - 1))
            h_sb = small.tile([128, 2 * C // 128], f32, tag="hsb",
                              name=f"h_{i}_{which}", bufs=2)
            nc.vector.tensor_tensor(out=h_sb, in0=hps, in1=fcb, op=OP.add)
            return h_sb

        def emit_stats(tag, sum3d, sq3d, h_sb, alpha):
            """AdaIN coefficients from accumulated sums: returns A, B, sinS, sinB."""
            # preload the sqrt ACT table set while the previous phase still runs
            nc.scalar.activation(out=junk, in_=eps_col, func=AF.Sqrt, bias=zero_col)
            sums = small.tile([128, G], f32, tag="sums", name=f"sums_{tag}")
            nc.vector.tensor_reduce(sums, sum3d, axis=AX.X, op=OP.add)
            sqs = small.tile([128, G], f32, tag="sqs", name=f"sqs_{tag}")
            nc.vector.tensor_reduce(sqs, sq3d, axis=AX.X, op=OP.add)
            mu = small.tile([128, G], f32, tag="mu", name=f"mu_{tag}")
            nc.vector.tensor_scalar(mu, sums, 1.0 / T, None, OP.mult)
            ex2 = small.tile([128, G], f32, tag="ex2", name=f"ex2_{tag}")
            nc.vector.tensor_scalar(ex2, sqs, 1.0 / T, None, OP.mult)
            var = small.tile([128, G], f32, tag="var", name=f"var_{tag}")
            nc.vector.tensor_tensor(out=var, in0=mu, in1=mu, op=OP.mult)
            nc.vector.tensor_tensor(out=var, in0=ex2, in1=var, op=OP.subtract)
            istd = small.tile([128, G], f32, tag="istd", name=f"istd_{tag}")
            nc.scalar.activation(out=istd, in_=var, func=AF.Sqrt, bias=eps_col)
            # preload the sin table set; hides behind the DVE coefficient chain
            nc.scalar.activation(out=junk, in_=eps_col, func=AF.Sin, bias=zero_col)
            nc.vector.reciprocal(istd, istd)
            A = small.tile([128, G], f32, tag="A", name=f"A_{tag}")
            nc.vector.tensor_scalar(A, h_sb[:, 0:G], 1.0, None, OP.add)  # 1+gamma
            nc.vector.tensor_tensor(out=A, in0=A, in1=istd, op=OP.mult)
            Bc = small.tile([128, G], f32, tag="Bc", name=f"B_{tag}")
            nc.vector.tensor_tensor(out=Bc, in0=mu, in1=A, op=OP.mult)
            nc.vector.tensor_tensor(out=Bc, in0=h_sb[:, G:2 * G], in1=Bc, op=OP.subtract)
            sinS = small.tile([128, G], f32, tag="sinS", name=f"sinS_{tag}")
            nc.vector.tensor_tensor(out=sinS, in0=A, in1=alpha, op=OP.mult)
            sinB = small.tile([128, G], f32, tag="sinB", name=f"sinB_{tag}")
            nc.vector.tensor_tensor(out=sinB, in0=Bc, in1=alpha, op=OP.mult)
            return A, Bc, sinS, sinB

        def make_snake(tag, src_fn, dst_fn, A, Bc, sinS, sinB, sqS):
            """dst = A*src + B + sin(alpha*(A*src+B))^2 / alpha, per group.

            ACT Sin is only valid on [-pi, pi], so the angle t = sinS*src+sinB
            is computed explicitly (gpsimd), range-wrapped by one period on
            the DVE, then Sin/Square run on ACT (in place) and
            affine_then_add fuses the final combine. Returns a per-chunk
            emitter so the caller can interleave snake chunks with the
            consuming conv's chunk loop — emitting the whole snake up front
            would park the conv's PSUM evictions behind every snake op in
            the in-order DVE queue and stall the PE on bank recycling."""
            PI = math.pi

            def emit_chunk(ic):
                c0, c1 = SNAKE_BOUNDS[ic], SNAKE_BOUNDS[ic + 1]
                csl = slice(c0, c1)
                cw = c1 - c0
                for g in range(G):
                    t_f = scr.tile([128, SNK_MAX], f16, tag=f"wrap{g}",
                                   name=f"wrap_{tag}_{g}_{ic}", bufs=3)
                    t_g = t_f[:, 0:cw]
                    nc.gpsimd.tensor_scalar(
                        t_g, src_fn(g)[:, csl],
                        sinS[:, g:g + 1], sinB[:, g:g + 1],
                        OP.mult, OP.add)
                    for _ in range(N_WRAPS):
                        nc.vector.add_range_wrap(
                            t_g, t_g, 0.0, PI, 2.0 * PI)
                    nc.scalar.activation(out=t_g, in_=t_g,
                                         func=AF.Sin, bias=zero_col)
                    nc.scalar.activation(out=t_g, in_=t_g,
                                         func=AF.Square,
                                         scale=sqS[:, g:g + 1], bias=zero_col)
                    nc.vector.affine_then_add(
                        out=dst_fn(g)[:, csl], in0=src_fn(g)[:, csl],
                        in1=t_g,
                        scale=A[:, g:g + 1], bias=Bc[:, g:g + 1])
            return emit_chunk

        # conv chunk tj needs snake chunk tj (520 >= 512+pad margin for all
        # tj). Pre-emit s0,s1; trickle the rest in with a 2-chunk lead via
        # conv fillers.
        SNAKE_SCHED = {tj: [tj + 2] for tj in range(len(SNAKE_BOUNDS) - 3)}

        def snake_filler(emit_chunk):
            def filler(tj):
                for s in SNAKE_SCHED.get(tj, []):
                    emit_chunk(s)
            return filler

        def emit_conv(tag, W, src_pad, pad, d, evict_fn, fillers=()):
            """Chunk-ordered conv: for each 512-col output chunk, 12 matmuls
            per co group accumulate one PSUM bank, then evict. Fillers run
            after each chunk to schedule next-conv weight prep mid-stream."""
            for tj in range(NT):
                for co in range(G):
                    pt = psc.tile([128, TCH], f32, tag="pc",
                                  name=f"ps_{tag}_{co}_{tj}")
                    n = 0
                    for ci in range(G):
                        for k in range(KW):
                            off = pad + tj * TCH + (k - 1) * d
                            nc.tensor.matmul(
                                pt,
                                W[k * G + ci][:, co * 128:(co + 1) * 128],
                                src_pad[ci][:, off:off + TCH],
                                start=(n == 0), stop=(n == KW * G - 1))
                            n += 1
                    evict_fn(co, tj, pt)
                for f in fillers:
                    f(tj)

        # ------------- prologue -------------
        # x pieces and the first conv-weight staging tiles interleave across
        # all three DMA queues so both finish together; the slow gather-style
        # param loads issue after them (they're not needed until the first
        # stats), keeping the gpsimd queue head free for bulk transfers.
        NP = 4
        PW = T // NP
        xsum_cur = small.tile([128, G, NP], f32, tag="xsum", name="xsum_in")
        xsq_cur = small.tile([128, G, NP], f32, tag="xsq", name="xsq_in")
        queues = [nc.sync, nc.scalar, nc.gpsimd]
        st01 = []
        for p in range(NP):
            sl = slice(p * PW, (p + 1) * PW)
            for g in range(G):
                queues[(p * G + g) % 3].dma_start(
                    out=x_cur[g][:, sl], in_=x_ext[g * 128:(g + 1) * 128, sl])
            for _ in range(3):
                vidx = len(st01)
                if vidx < KW * G:
                    k, ci = divmod(vidx, G)
                    st_v = stage.tile([128, TCH], f32, tag="vstg", bufs=10,
                                      name=f"vst_0_1_{k}_{ci}")
                    queues[vidx % 3].dma_start(
                        out=st_v,
                        in_=conv1_v_ext[0, k, ci * 128:(ci + 1) * 128, :])
                    st01.append(st_v)

        for g in range(G):
            nc.gpsimd.memset(b1pad[g][:, 0:PADL], 0.0)
            nc.gpsimd.memset(b1pad[g][:, PADL + T:PADL + T + PADL], 0.0)
            nc.gpsimd.memset(cb2pad[g][:, 0:1], 0.0)
            nc.gpsimd.memset(cb2pad[g][:, 1 + T:1 + T + 1], 0.0)

        alpha1_all = load_pcvec_all("alpha1_all", alpha1_ext)
        alpha2_all = load_pcvec_all("alpha2_all", alpha2_ext)
        g1_all = load_pcvec_all("g1_all", conv1_g_ext)
        g2_all = load_pcvec_all("g2_all", conv2_g_ext)
        cb2_all = load_pcvec_all("cb2_all", conv2_b_ext)
        alpha_t, g_t, cb_t = {}, {}, {}
        for i in range(3):
            alpha_t[(i, 1)] = alpha1_all[:, i, :]
            alpha_t[(i, 2)] = alpha2_all[:, i, :]
            g_t[(i, 1)] = g1_all[:, i, :]
            g_t[(i, 2)] = g2_all[:, i, :]
            cb_t[(i, 2)] = cb2_all[:, i, :]

        for p in range(NP):
            sl = slice(p * PW, (p + 1) * PW)
            for g in range(G):
                nc.vector.tensor_scalar(
                    cb2pad[g][:, 1 + p * PW:1 + (p + 1) * PW], x_cur[g][:, sl],
                    0.0, None, OP.add, OP.add,
                    accum_out=xsum_cur[:, g, p:p + 1])
                nc.scalar.activation(
                    out=b1pad[g][:, PADL + p * PW:PADL + (p + 1) * PW],
                    in_=x_cur[g][:, sl], func=AF.Square, bias=zero_col,
                    accum_out=xsq_cur[:, g, p:p + 1])

        # alpha -> 1/sqrt(alpha), batched over all 3 layers (emitted after the
        # piece stats so the ACT queue isn't head-blocked waiting for alphas)
        sqS_t = {}
        for which, a_all in ((1, alpha1_all), (2, alpha2_all)):
            sq = persist.tile([128, 3, G], f32, name=f"sqS{which}_all2")
            nc.scalar.activation(out=sq, in_=a_all, func=AF.Sqrt, bias=zero_col)
            nc.vector.reciprocal(sq, sq)          # 1/sqrt(alpha)
            for i in range(3):
                sqS_t[(i, which)] = sq[:, i, :]

        # conv weights for layer 0 conv1 prepped immediately (PE is idle);
        # subsequent sets ride the conv fillers.
        W1_cur = [None] * (KW * G)
        wv1 = []
        emit_wprep_cast(0, 1, st01, 0, KW * G, W1_cur, wv1)
        wnS1_cur = emit_wprep_norm(0, 1, wv1)
        W2_cur, wnS2_box = None, None

        # ------------- iterations -------------
        pending_bias = None
        next_c1_fillers = []
        for i in range(n_iters):
            d = DILATIONS[i]
            last = (i == n_iters - 1)
            h1 = emit_fc(i, 1)
            h2 = emit_fc(i, 2)

            # conv2 bias is a per-channel constant: every downstream consumer
            # of x except the final output is an instance norm (which absorbs
            # it) or the residual chain. Accumulate it; fold into x once, via
            # fillers under the final conv1.
            if pending_bias is None:
                pending_bias = small.tile([128, G], f32, tag="pend",
                                          name="pending_bias", bufs=1)
                nc.vector.tensor_copy(pending_bias, cb_t[(i, 2)])
            else:
                nc.vector.tensor_tensor(out=pending_bias, in0=pending_bias,
                                        in1=cb_t[(i, 2)], op=OP.add)

            A1, B1, sinS1, sinB1 = emit_stats(
                f"a1_{i}", xsum_cur, xsq_cur, h1, alpha_t[(i, 1)])
            snake1 = make_snake(
                f"s1_{i}",
                src_fn=lambda g: x_cur[g][:, 0:T],
                dst_fn=lambda g: b1pad[g][:, PADL:PADL + T],
                A=A1, Bc=B1, sinS=sinS1, sinB=sinB1, sqS=sqS_t[(i, 1)])
            snake1(0)
            snake1(1)

            c1sum = small.tile([128, G, NT], f32, tag="c1sum", name=f"c1sum_{i}")
            c1sq = small.tile([128, G, NT], f32, tag="c1sq", name=f"c1sq_{i}")

            # conv1 bias is a per-channel constant absorbed exactly by the
            # following instance norm, so it is not applied at all. The
            # sum-of-squares reads straight from PSUM (scaled inside the ACT
            # op) so it doesn't serialize behind the DVE eviction.
            def evict1(co, tj, pt, c1sum=c1sum, c1sq=c1sq, wnS1=wnS1_cur):
                dst = cb2pad[co][:, 1 + tj * TCH: 1 + (tj + 1) * TCH]
                nc.vector.tensor_scalar(
                    dst, pt, wnS1[:, co:co + 1], None, OP.mult, OP.add,
                    accum_out=c1sum[:, co, tj:tj + 1])
                # the sum-of-squares splits across ACT (reading PSUM, scale
                # fused) and DVE (affine_mul_reduce on the evicted chunk) —
                # ACT alone is the saturated engine inside the convs
                if co < 2:
                    nc.vector.affine_mul_reduce(
                        out=sqjunk2, accum_out=c1sq[:, co, tj:tj + 1],
                        in0=dst, in1=dst, scale=1.0, bias=0.0)
                else:
                    nc.scalar.activation(
                        out=sqjunk, in_=pt, func=AF.Square,
                        scale=wnS1[:, co:co + 1], bias=zero_col,
                        accum_out=c1sq[:, co, tj:tj + 1])

            def preload_sqrt_filler(tj):
                if tj == 6:
                    nc.scalar.activation(out=junk, in_=eps_col, func=AF.Sqrt,
                                         bias=zero_col)

            fillers1 = [snake_filler(snake1)] + list(next_c1_fillers) + \
                [preload_sqrt_filler]
            next_c1_fillers = []
            if W2_cur is None:
                f2, W2_cur, wnS2_box = wprep_fillers(0, 2)
                fillers1.append(f2)
            if last:
                # fold the accumulated conv2 biases into x chunk-by-chunk
                # under conv1, before the final conv2's residual eviction
                # adds on top (snake1 has finished reading each chunk by
                # the time its filler runs).
                def bias_filler(tj, pb=pending_bias):
                    for g in range(G):
                        nc.scalar.activation(
                            out=x_cur[g][:, tj * TCH:(tj + 1) * TCH],
                            in_=x_cur[g][:, tj * TCH:(tj + 1) * TCH],
                            func=AF.Identity,
                            bias=pb[:, g:g + 1], scale=1.0)
                fillers1.append(bias_filler)
            emit_conv(f"c1_{i}", W1_cur, b1pad, PADL, d, evict1,
                      fillers=fillers1)

            A2, B2, sinS2, sinB2 = emit_stats(
                f"a2_{i}", c1sum, c1sq, h2, alpha_t[(i, 2)])
            snake2 = make_snake(
                f"s2_{i}",
                src_fn=lambda g: cb2pad[g][:, 1:1 + T],
                dst_fn=lambda g: cb2pad[g][:, 1:1 + T],
                A=A2, Bc=B2, sinS=sinS2, sinB=sinB2, sqS=sqS_t[(i, 2)])
            snake2(0)
            snake2(1)

            xsum_nxt = small.tile([128, G, NT], f32, tag="xsum", name=f"xsum_{i}")
            xsq_nxt = small.tile([128, G, NT], f32, tag="xsq", name=f"xsq_{i}")
            wnS2_cur = wnS2_box[0]

            def evict2(co, tj, pt, wnS2=wnS2_cur, xsum_nxt=xsum_nxt,
                       xsq_nxt=xsq_nxt, last=last):
                sl = x_cur[co][:, tj * TCH:(tj + 1) * TCH]
                nc.vector.scalar_tensor_tensor(
                    out=sl, in0=pt, scalar=wnS2[:, co:co + 1], in1=sl,
                    op0=OP.mult, op1=OP.add,
                    accum_out=xsum_nxt[:, co, tj:tj + 1])
                if last:
                    (nc.scalar if (co + tj) % 2 else nc.sync).dma_start(
                        out=out_ext[co * 128:(co + 1) * 128,
                                    tj * TCH:(tj + 1) * TCH], in_=sl)
                elif co < 2:
                    nc.vector.affine_mul_reduce(
                        out=sqjunk2, accum_out=xsq_nxt[:, co, tj:tj + 1],
                        in0=sl, in1=sl, scale=1.0, bias=0.0)
                else:
                    nc.scalar.activation(
                        out=sqjunk, in_=sl, func=AF.Square, bias=zero_col,
                        accum_out=xsq_nxt[:, co, tj:tj + 1])

            fillers2 = [snake_filler(snake2)]
            if not last:
                f1n, W1_nxt, wnS1_box = wprep_fillers(i + 1, 1)
                fillers2.append(f1n)
                fillers2.append(preload_sqrt_filler)
            emit_conv(f"c2_{i}", W2_cur, cb2pad, 1, 1, evict2,
                      fillers=fillers2)
            xsum_cur, xsq_cur = xsum_nxt, xsq_nxt

            if not last:
                wnS1_cur = wnS1_box[0]
                W1_cur = W1_nxt
                f2n, W2_cur, wnS2_box = wprep_fillers(i + 1, 2)
                next_c1_fillers.append(f2n)

    return nc


def make_in_maps(inputs, T=T_FULL):
    npf = lambda v: np.asarray(v, dtype=np.float32)
    x = npf(inputs["x"])
    s = npf(inputs["s"])
    shared = {
        "fc1_w": npf(inputs["fc1_w"]),
        "fc1_b": npf(inputs["fc1_b"]),
        "alpha1": npf(inputs["alpha1"]).reshape(3, C),
        "conv1_v": npf(inputs["conv1_v"]),
        "conv1_g": npf(inputs["conv1_g"]),
        "conv1_b": npf(inputs["conv1_b"]),
        "fc2_w": npf(inputs["fc2_w"]),
        "fc2_b": npf(inputs["fc2_b"]),
        "alpha2": npf(inputs["alpha2"]).reshape(3, C),
        "conv2_v": npf(inputs["conv2_v"]),
        "conv2_g": npf(inputs["conv2_g"]),
        "conv2_b": npf(inputs["conv2_b"]),
    }
    in_maps = []
    for b in range(N_CORES):
        m = dict(shared)
        m["x"] = np.ascontiguousarray(x[b, :T, :].T)
        m["s"] = np.ascontiguousarray(s[b].reshape(S, 1))
        in_maps.append(m)
    return in_maps


_CACHED = {}


def kernel(**inputs) -> np.ndarray:
    from concourse.bass_utils import run_bass_kernel_spmd

    max_alpha = float(max(np.abs(np.asarray(inputs["alpha1"])).max(),
                          np.abs(np.asarray(inputs["alpha2"])).max()))
    key = ("nc", max_alpha)
    if key not in _CACHED:
        nc = build_nc(T_FULL, max_alpha=max_alpha)
        nc.finalize()
        _CACHED[key] = nc
    nc = _CACHED[key]
    in_maps = make_in_maps(inputs, T_FULL)
    res = run_bass_kernel_spmd(nc, in_maps, core_ids=list(range(N_CORES)))
    out = np.stack(
        [np.asarray(res.results[i]["out"]).T for i in range(N_CORES)], axis=0)
    return np.ascontiguousarray(out).astype(np.float32)
